# revision 1
# baseline (speedup 1.0000x reference)
"""Trainium2 Bass kernel for EnrichAttention (attention block + GRU).

Contract: kernel(**inputs) takes FULL unsharded inputs (see shapes below),
shards batch B=32 across 8 NeuronCores (4 sequences per core, weights
replicated), runs one SPMD NEFF, and returns the full [32, 512, 256] output.

Self-contained: hardcodes all shapes; no sibling imports.
"""

import numpy as np

# Problem shapes (hardcoded per contract)
B, L, H, A = 32, 512, 256, 256
NCORES = 8
BL = B // NCORES          # 4 sequences per core
P = 128                   # SBUF partitions
HC = H // P               # 2 hidden chunks
AC = A // P               # 2
GC = (2 * H) // P         # 4 chunks of GRU input dim
UC = (3 * H) // P         # 6 chunks of gate dim (768)
U = 3 * H                 # 768
G = 2 * H                 # 512
PHASE_A_BF16 = False      # run attention/xp matmuls in bf16 (2x PE rate + FWL)


def _body(tc, outs, ins, T):
    import concourse.bass as bass
    import concourse.mybir as mybir
    from concourse.masks import make_identity

    f32 = mybir.dt.float32
    bf16 = mybir.dt.bfloat16
    Alu = mybir.AluOpType
    Act = mybir.ActivationFunctionType

    nc = tc.nc
    dout = outs[0]
    (dx1, dx2, dw1, dw2, dD, dW, dwih, dwhh, dbih, dbhh) = ins

    f32r = mybir.dt.float32r
    mmdt = bf16 if PHASE_A_BF16 else f32r

    def mm32r(out, lhsT, rhs, start, stop):
        # phase-A matmul: bf16 (fast) or fp32 via replicated-bf16 path
        nc.tensor.matmul(out, lhsT, rhs, start=start, stop=stop)

    from contextlib import ExitStack

    with ExitStack() as stk:
        const = stk.enter_context(tc.tile_pool(name="const", bufs=1))
        stW = stk.enter_context(ExitStack())
        wtmp = stW.enter_context(tc.tile_pool(name="wtmp", bufs=1))
        psW = stW.enter_context(tc.tile_pool(name="psW", bufs=4, space="PSUM"))

        ident = const.tile([P, P], f32)
        make_identity(nc, ident)

        # ---- weight prep: transposes via PE ----
        def load_nat(dram, rows, cols, tag):
            # dram [rows, cols] -> sbuf [P, rows//P, cols]
            t = wtmp.tile([P, rows // P, cols], f32, tag=tag)
            nc.sync.dma_start(out=t, in_=dram.rearrange("(c p) h -> p c h", p=P))
            return t

        def transpose_into(dst, dst_c, dst_off, src, src_c, src_off, cast=False):
            # dst[:, dst_c, dst_off:dst_off+P] = src[:, src_c, src_off:src_off+P].T
            tr = psW.tile([P, P], f32, tag="psWtr")
            nc.tensor.transpose(tr, src[:, src_c, src_off:src_off + P], ident)
            nc.vector.tensor_copy(dst[:, dst_c, dst_off:dst_off + P], tr)

        w1n = load_nat(dw1, A, H, "w1n")
        w2n = load_nat(dw2, A, H, "w2n")
        w1T = const.tile([P, HC, A], mmdt)
        w2T = const.tile([P, HC, A], mmdt)
        for c in range(AC):
            for hc in range(HC):
                transpose_into(w1T, hc, c * P, w1n, c, hc * P)
                transpose_into(w2T, hc, c * P, w2n, c, hc * P)

        Dn_raw = wtmp.tile([P, AC, A], f32, tag="Dn_raw")
        nc.sync.dma_start(out=Dn_raw, in_=dD.rearrange("(c p) h -> p c h", p=P))
        Dn = const.tile([P, AC, A], mmdt)
        nc.vector.tensor_copy(Dn, Dn_raw)

        Wn = load_nat(dW, L, L, "Wn")
        WTt = const.tile([P, 4, L], f32)
        for it in range(4):
            for jt in range(4):
                transpose_into(WTt, jt, it * P, Wn, it, jt * P)

        wihn = load_nat(dwih, U, G, "wihn")
        wihT = const.tile([P, GC, U], mmdt)
        for uc in range(UC):
            for gc in range(GC):
                transpose_into(wihT, gc, uc * P, wihn, uc, gc * P)

        whhn = load_nat(dwhh, U, H, "whhn")
        whhTb = const.tile([P, HC, U], bf16)
        for uc in range(UC):
            for kc in range(HC):
                tr = psW.tile([P, P], f32, tag="psWtr")
                nc.tensor.transpose(tr, whhn[:, uc, kc * P:kc * P + P], ident)
                nc.vector.tensor_copy(whhTb[:, kc, uc * P:uc * P + P], tr)

        bihv = const.tile([P, UC], f32)
        bhhv = const.tile([P, UC], f32)
        nc.sync.dma_start(out=bihv, in_=dbih.rearrange("(c p) -> p c", p=P))
        nc.sync.dma_start(out=bhhv, in_=dbhh.rearrange("(c p) -> p c", p=P))
        bias_x = const.tile([P, UC], f32)
        # r,z gates: xp bias = bih + bhh (both fold); n gate: bih only
        nc.vector.tensor_add(bias_x[:, 0:4], bihv[:, 0:4], bhhv[:, 0:4])
        nc.vector.tensor_copy(bias_x[:, 4:6], bihv[:, 4:6])

        BG = BL // 2  # sequences per interleave group
        zerob = const.tile([P, HC, BG], bf16)
        zerof = const.tile([P, HC, BG], f32)
        nc.vector.memset(zerob, 0.0)
        nc.vector.memset(zerof, 0.0)
        identb = const.tile([P, P], bf16)
        nc.vector.tensor_copy(identb, ident)
        # persistent activations
        # strips 0-3: r/z input projection (+both biases); strips 4-5: bhh_n
        # broadcast over (b, t) so one identity matmul injects everything.
        xprz = const.tile([P, 6, BL, L], bf16)
        xpn = const.tile([P, 2, BL, L], f32)         # n-gate input projection (+bih)
        for s in range(2):
            nc.vector.tensor_scalar_add(
                xprz[:, 4 + s, :, :].rearrange("p b l -> p (b l)"),
                zerof[:, 0, 0:1].to_broadcast((P, BL * L)),
                bhhv[:, 4 + s:5 + s])
        out_stage = const.tile([P, HC, BL, L], f32)  # GRU hidden states (transposed)

        stW.close()  # free weight-prep temporaries

        # ---- phase A: attention + xp precompute, per local batch ----
        with ExitStack() as stA:
            pA = stA.enter_context(tc.tile_pool(name="pA", bufs=2))
            pT = stA.enter_context(tc.tile_pool(name="pT", bufs=2))
            pS = stA.enter_context(tc.tile_pool(name="pS", bufs=4))
            psA = stA.enter_context(tc.tile_pool(name="psA", bufs=3, space="PSUM"))
            psT = stA.enter_context(tc.tile_pool(name="psT", bufs=4, space="PSUM"))

            for b in range(BL):
                x1n = pA.tile([P, 4, H], f32, tag="x1n")
                x2n = pA.tile([P, 4, H], f32, tag="x2n")
                nc.sync.dma_start(out=x1n, in_=dx1[b].rearrange("(it p) h -> p it h", p=P))
                nc.sync.dma_start(out=x2n, in_=dx2[b].rearrange("(it p) h -> p it h", p=P))

                x1T = pT.tile([P, HC, L], mmdt, tag="x1T")
                x2T = pT.tile([P, HC, L], mmdt, tag="x2T")
                for it in range(4):
                    for hc in range(HC):
                        tr = psT.tile([P, P], f32, tag="psTtr")
                        nc.tensor.transpose(tr, x1n[:, it, hc * P:hc * P + P], ident)
                        nc.vector.tensor_copy(x1T[:, hc, it * P:it * P + P], tr)
                        tr2 = psT.tile([P, P], f32, tag="psTtr")
                        nc.tensor.transpose(tr2, x2n[:, it, hc * P:hc * P + P], ident)
                        nc.vector.tensor_copy(x2T[:, hc, it * P:it * P + P], tr2)

                # a1T = relu(w1 @ x1T), a2T = relu(w2 @ x2T): [P, AC, L]
                a1T = pT.tile([P, AC, L], mmdt, tag="a1T")
                a2T = pT.tile([P, AC, L], mmdt, tag="a2T")
                for ac in range(AC):
                    ps = psA.tile([P, L], f32, tag="psAmm")
                    for hc in range(HC):
                        mm32r(ps, w1T[:, hc, ac * P:ac * P + P],
                              x1T[:, hc, :], start=(hc == 0), stop=(hc == HC - 1))
                    nc.scalar.activation(a1T[:, ac, :], ps, Act.Relu)
                    ps2 = psA.tile([P, L], f32, tag="psAmm")
                    for hc in range(HC):
                        mm32r(ps2, w2T[:, hc, ac * P:ac * P + P],
                              x2T[:, hc, :], start=(hc == 0), stop=(hc == HC - 1))
                    nc.scalar.activation(a2T[:, ac, :], ps2, Act.Relu)

                # a1DT[a', i] = sum_a D[a, a'] * a1T[a, i]
                a1DT = pT.tile([P, AC, L], mmdt, tag="a1DT")
                for pc in range(AC):
                    ps = psA.tile([P, L], f32, tag="psAmm")
                    for ac in range(AC):
                        mm32r(ps, Dn[:, ac, pc * P:pc * P + P],
                              a1T[:, ac, :], start=(ac == 0), stop=(ac == AC - 1))
                    nc.vector.tensor_copy(a1DT[:, pc, :], ps)

                # MT[j, i] = sum_a a2T[a, j] * a1DT[a, i]; softmax over i (free dim)
                MTn = pT.tile([P, 4, L], mmdt, tag="MTn")
                for jt in range(4):
                    ps = psA.tile([P, L], f32, tag="psAmm")
                    for ac in range(AC):
                        mm32r(ps, a2T[:, ac, jt * P:jt * P + P],
                              a1DT[:, ac, :], start=(ac == 0), stop=(ac == AC - 1))
                    tw = pS.tile([P, L], f32, tag="tw")
                    nc.vector.tensor_tensor(tw, ps, WTt[:, jt, :], Alu.mult)
                    ssum = pS.tile([P, 1], f32, tag="ssum")
                    nc.scalar.activation(MTn[:, jt, :], tw, Act.Exp, accum_out=ssum)
                    rv = pS.tile([P, 1], f32, tag="rv")
                    nc.vector.reciprocal(rv, ssum)
                    nc.vector.tensor_scalar_mul(MTn[:, jt, :], MTn[:, jt, :], rv)

                x2r = pA.tile([P, 4, H], mmdt, tag="x2r")
                nc.vector.tensor_copy(x2r, x2n)
                # ctxT[h, i] = sum_j x2[j, h] * MTn[j, i]
                ctxT = pT.tile([P, HC, L], mmdt, tag="ctxT")
                for hc in range(HC):
                    ps = psA.tile([P, L], f32, tag="psAmm")
                    for jc in range(4):
                        mm32r(ps, x2r[:, jc, hc * P:hc * P + P],
                              MTn[:, jc, :], start=(jc == 0), stop=(jc == 3))
                    nc.vector.tensor_copy(ctxT[:, hc, :], ps)

                # xp[u, b, i] = sum_g wih[u, g] gT[g, i] + bias
                for uc in range(UC):
                    ps = psA.tile([P, L], f32, tag="psAmm")
                    for gc in range(GC):
                        rhs = x1T[:, gc, :] if gc < HC else ctxT[:, gc - HC, :]
                        mm32r(ps, wihT[:, gc, uc * P:uc * P + P],
                              rhs, start=(gc == 0), stop=(gc == GC - 1))
                    dst = xprz[:, uc, b, :] if uc < 4 else xpn[:, uc - 4, b, :]
                    nc.scalar.activation(dst, ps, Act.Identity,
                                         bias=bias_x[:, uc:uc + 1])

        # ---- phase B: the GRU recurrence (T serial steps) ----
        # Two interleaved independent chains (sequences 0-1 and 2-3) so the
        # serial chain latency of one hides behind engine throughput of the
        # other. xp_rz and bhh_n are pre-injected into PSUM by identity
        # matmuls, so sigmoid reads PSUM directly. Output untransposing
        # (phase C) is interleaved into the loop every 128 steps to use the
        # PE/DVE slack during the latency-bound recurrence.
        with ExitStack() as stB:
            pg = stB.enter_context(tc.tile_pool(name="pg", bufs=3))
            psg = [
                stB.enter_context(tc.tile_pool(name=f"ps_h{g}", bufs=2, space="PSUM"))
                for g in range(2)
            ]
            pC = stB.enter_context(tc.tile_pool(name="pC", bufs=2))
            psC = stB.enter_context(tc.tile_pool(name="psC", bufs=2, space="PSUM"))

            def emit_out_quarter(tt_i, tn):
                for b in range(BL):
                    ob = pC.tile([P, H], f32, tag="ob")
                    for kc in range(HC):
                        tr = psC.tile([P, P], f32, tag="psCtr")
                        nc.tensor.transpose(
                            tr[:tn, :],
                            out_stage[:, kc, b, tt_i * P:tt_i * P + tn], ident)
                        nc.vector.tensor_copy(ob[:tn, kc * P:kc * P + P], tr[:tn, :])
                    nc.sync.dma_start(out=dout[b, tt_i * P:tt_i * P + tn, :],
                                      in_=ob[:tn, :])

            hbf_prev = [zerob, zerob]
            hf_prev = [zerof, zerof]
            for t in range(T):
                halls = []
                for g in range(2):
                    bs = slice(g * BG, (g + 1) * BG)
                    hall = psg[g].tile([P, 6, BG], f32, tag=f"hall{g}")
                    # single inject: xp_rz (+biases) and bhh_n into psum;
                    # both groups' injects adjacent -> ident loads pipeline
                    nc.tensor.matmul(hall, identb, xprz[:, :, bs, t],
                                     start=True, stop=False, skip_group_check=True)
                    halls.append(hall)
                for g in range(2):
                    bs = slice(g * BG, (g + 1) * BG)
                    hall = halls[g]
                    hrz = hall[:, 0:4, :]
                    hn = hall[:, 4:6, :]
                    # accumulate h_{t-1} @ whh.T
                    for uc in range(UC):
                        dst = hrz[:, uc, :] if uc < 4 else hn[:, uc - 4, :]
                        for kc in range(HC):
                            nc.tensor.matmul(dst, whhTb[:, kc, uc * P:uc * P + P],
                                             hbf_prev[g][:, kc, :],
                                             start=False, stop=(kc == HC - 1),
                                             skip_group_check=True)

                    rz = pg.tile([P, 4, BG], f32, tag=f"rz{g}")
                    nc.scalar.activation(rz, hrz, Act.Sigmoid)
                    pzh = pg.tile([P, 2, BG], f32, tag=f"pzh{g}")
                    nc.gpsimd.tensor_tensor(pzh, rz[:, 2:4, :], hf_prev[g], Alu.mult)

                    # n = tanh(xp_n + r * (hp_n + bhh_n))
                    tt = pg.tile([P, 2, BG], f32, tag=f"tt{g}")
                    nc.vector.tensor_tensor(tt, rz[:, 0:2, :], hn, Alu.mult)
                    u = pg.tile([P, 2, BG], f32, tag=f"u{g}")
                    nc.vector.tensor_tensor(u, tt, xpn[:, :, bs, t], Alu.add)
                    nn_ = pg.tile([P, 2, BG], f32, tag=f"nn{g}")
                    nc.scalar.activation(nn_, u, Act.Tanh)

                    # h' = n*(1-z) + z*h, via q' = (z-1)*n and h' = z*h - q'
                    q = pg.tile([P, 2, BG], f32, tag=f"q{g}")
                    nc.vector.scalar_tensor_tensor(q, rz[:, 2:4, :], 1.0, nn_,
                                                   Alu.subtract, Alu.mult)
                    hbf = pg.tile([P, HC, BG], bf16, tag=f"hbf{g}")
                    nc.vector.tensor_tensor(hbf, pzh, q, Alu.subtract)
                    nc.gpsimd.tensor_tensor(out_stage[:, :, bs, t], pzh, q, Alu.subtract)
                    hbf_prev[g] = hbf
                    hf_prev[g] = out_stage[:, :, bs, t]

                if t % P == P - 1:
                    emit_out_quarter(t // P, P)
            if T % P != 0:
                emit_out_quarter(T // P, T % P)


def build(T=L):
    """Build and compile the per-core Bass program. Returns nc."""
    import concourse.mybir as mybir
    import concourse.tile as tile
    from concourse import bacc

    f32 = mybir.dt.float32
    nc = bacc.Bacc("TRN2", target_bir_lowering=False, debug=False)
    dx1 = nc.dram_tensor("x1", [BL, L, H], f32, kind="ExternalInput").ap()
    dx2 = nc.dram_tensor("x2", [BL, L, H], f32, kind="ExternalInput").ap()
    dw1 = nc.dram_tensor("w1", [A, H], f32, kind="ExternalInput").ap()
    dw2 = nc.dram_tensor("w2", [A, H], f32, kind="ExternalInput").ap()
    dD = nc.dram_tensor("D", [A, A], f32, kind="ExternalInput").ap()
    dW = nc.dram_tensor("W", [L, L], f32, kind="ExternalInput").ap()
    dwih = nc.dram_tensor("gru_wih", [U, G], f32, kind="ExternalInput").ap()
    dwhh = nc.dram_tensor("gru_whh", [U, H], f32, kind="ExternalInput").ap()
    dbih = nc.dram_tensor("gru_bih", [U], f32, kind="ExternalInput").ap()
    dbhh = nc.dram_tensor("gru_bhh", [U], f32, kind="ExternalInput").ap()
    dout = nc.dram_tensor("out", [BL, T, H], f32, kind="ExternalOutput").ap()

    ins = (dx1, dx2, dw1, dw2, dD, dW, dwih, dwhh, dbih, dbhh)
    with tile.TileContext(nc) as tc:
        _body(tc, [dout], ins, T)
    nc.compile()
    return nc


_NC_CACHE = {}


def kernel(**inputs):
    import concourse.bass_utils as bass_utils

    if "nc" not in _NC_CACHE:
        _NC_CACHE["nc"] = build(L)
    nc = _NC_CACHE["nc"]

    names = ["x1", "x2", "w1", "w2", "D", "W", "gru_wih", "gru_whh", "gru_bih", "gru_bhh"]
    full = {k: np.ascontiguousarray(np.asarray(inputs[k], dtype=np.float32)) for k in names}
    in_maps = []
    for c in range(NCORES):
        m = dict(full)
        m["x1"] = np.ascontiguousarray(full["x1"][c * BL:(c + 1) * BL])
        m["x2"] = np.ascontiguousarray(full["x2"][c * BL:(c + 1) * BL])
        in_maps.append(m)

    res = bass_utils.run_bass_kernel_spmd(nc, in_maps, core_ids=list(range(NCORES)))
    out = np.concatenate([r["out"] for r in res.results], axis=0)
    return out

